# revision 54
# baseline (speedup 1.0000x reference)
"""Trainium2 Bass kernel: MultiHeadSelfAttention (LayerNorm -> QKV -> masked
softmax attention -> output projection).

Problem shapes: B=4, S=2048, D=512, H=8, DK=64, fp32 I/O.

Sharding: 8 cores = 4 batches x 2 query-halves. Each core computes the full
K/V for its batch and attention outputs for its 1024 queries; no cross-core
communication. SPMD trick: the token order of each core's input is permuted on
the host so that the core's queries are always tokens 0..1023 (one static
program for all cores; attention is permutation-equivariant over keys as long
as the key-padding mask is permuted consistently).

Performance structure (v2): the exp of the masked scores on the Scalar engine
(ACT) is the hard floor (~143us of ACTIVATE). Everything else is arranged so
ACT is never starved:
  - front phase engine balance: LN stats on DVE, LN apply + weight casts on
    GpSimd, xnT/Q evacuations + LN sqrt on ACT, K/V evacuations on DVE, big
    1MB batched DMAs (x on the sync queue ring, weights on the gpsimd ring).
  - attention: per (pair, chunk): 4 scores matmuls (K=64 row-tiled), 2 exp
    ACTIVATEs [128,1024] (scale+mask+exp fused), PV of the previous chunk
    with the exp'd scores as matmul stationary into 3 packed PSUM banks.
    With scp bufs=2 the PSUM rotation exactly matches ACT's pipeline.
  - biases are all zero for this module (beta=0, b*=0); the zero-bias build
    skips them entirely (a general build with biases is kept as fallback).
"""

import math

import numpy as np

import concourse.bass as bass
import concourse.tile as tile
from concourse import bacc, mybir
from concourse.bass_utils import run_bass_kernel_spmd
from concourse.masks import make_identity

B, S, D, H, DK = 4, 2048, 512, 8, 64
P = 128                 # partitions
NQ = 1024               # queries per core
NT = S // P             # 16 token tiles / key chunks
DC = D // P             # 4 d-chunks
NQT = NQ // P           # 8 query tiles
PAIRS = H // 2          # 4 head pairs
F32 = mybir.dt.float32
BF16 = mybir.dt.bfloat16
NEG = -1.0e30
# Global shift applied inside exp (via the mask bias): keeps exp(score+SHIFT)
# within fp8e4m3 range (max 448). Max score measured ~9.7 for this module;
# exp(9.7-4) = 298. Scores below -2.24 flush to zero (< 3e-5 relative weight).
ESHIFT = -4.0


def _emit(tc: tile.TileContext, ctx, use_bias: bool):
    nc = tc.nc

    x_d = nc.dram_tensor("x", [S, D], F32, kind="ExternalInput")
    wq_d = nc.dram_tensor("wq", [D, D], BF16, kind="ExternalInput")
    wk_d = nc.dram_tensor("wk", [D, D], BF16, kind="ExternalInput")
    wv_d = nc.dram_tensor("wv", [D, D], BF16, kind="ExternalInput")
    wo_d = nc.dram_tensor("wo", [D, D], BF16, kind="ExternalInput")
    mb_d = nc.dram_tensor("maskb", [P, NT], F32, kind="ExternalInput")
    y_d = nc.dram_tensor("y", [NQ, D], F32, kind="ExternalOutput")
    if use_bias:
        bq_d = nc.dram_tensor("bq", [P, DC], F32, kind="ExternalInput")
        bk_d = nc.dram_tensor("bk", [P, DC], F32, kind="ExternalInput")
        bo_d = nc.dram_tensor("bo", [D], F32, kind="ExternalInput")

    consts = ctx.enter_context(tc.tile_pool(name="consts", bufs=1))
    big = ctx.enter_context(tc.tile_pool(name="big", bufs=1))
    xstage = ctx.enter_context(tc.tile_pool(name="xstage", bufs=4))
    xnp = ctx.enter_context(tc.tile_pool(name="xnp", bufs=3))
    stats = ctx.enter_context(tc.tile_pool(name="stats", bufs=8))
    ptp = ctx.enter_context(tc.tile_pool(name="ptp", bufs=3))
    rlp = ctx.enter_context(tc.tile_pool(name="rlp", bufs=6))
    yout = ctx.enter_context(tc.tile_pool(name="yout", bufs=3))

    ident = consts.tile([P, P], BF16, tag="ident")
    make_identity(nc, ident)
    mb_sb = consts.tile([P, NT], F32, tag="mb")
    nc.sync.dma_start(mb_sb, mb_d[:, :])
    eps_sb = consts.tile([P, 1], F32, tag="eps")
    nc.vector.memset(eps_sb, 1e-5)
    if use_bias:
        bq_sb = consts.tile([P, DC], F32, tag="bq")
        nc.sync.dma_start(bq_sb, bq_d[:, :])
        bk_sb = consts.tile([P, DC], F32, tag="bk")
        nc.sync.dma_start(bk_sb, bk_d[:, :])
        bo_sb = consts.tile([P, D], F32, tag="bo")
        bo_ap = bo_d[:]
        nc.sync.dma_start(
            bo_sb, bass.AP(tensor=bo_ap.tensor, offset=bo_ap.offset, ap=[[0, P], [1, D]])
        )

    # persistent bf16 operands
    w_sb = {}
    for name in ("wq", "wk", "wv", "wo"):
        w_sb[name] = big.tile([P, DC, D], BF16, tag=f"{name}_sb", name=f"{name}_sb")
    xnT = big.tile([P, DC, S], BF16, tag="xnT")
    # query operand zero-padded per head: [pair, head-slot, q]; the scores
    # matmul then contracts over the full 128 partitions (both heads of the
    # stationary kT) with the other head's rows zeroed -> full-array mode,
    # no 64-row tiling, no PE mode switches in the attention loop.
    qTp = big.tile([P, PAIRS, 2, NQ], BF16, tag="qTp")
    kT = big.tile([P, DC, S], BF16, tag="kT")
    vaug = big.tile([P, NT, 8 * 65], BF16, tag="vaug")
    attno = big.tile([P, NQT, D], BF16, tag="attno")
    outT = big.tile([P, DC, NQ], BF16, tag="outT")

    # ---------------- weight + x loads ----------------
    # weights are bf16 on the host already -> DMA straight into w_sb on the
    # scalar engine's HWDGE ring (no staging, no casts); x on the sync ring.
    def dma_w(name):
        d = {"wq": wq_d, "wk": wk_d, "wv": wv_d, "wo": wo_d}[name]
        dap = d[:, :]
        nc.scalar.dma_start(
            w_sb[name],
            bass.AP(
                tensor=dap.tensor, offset=dap.offset,
                ap=[[D, P], [P * D, DC], [1, D]],
            ),
        )

    dma_w("wq")
    dma_w("wk")
    dma_w("wv")
    dma_w("wo")
    # dependency-free zero-fills while the DVE waits for the first x tile
    nc.vector.memset(qTp[0:DK, :, 1, :], 0.0)
    nc.vector.memset(qTp[DK:P, :, 0, :], 0.0)
    nc.vector.memset(
        vaug[:, :, :].rearrange("p t (h c) -> p t h c", h=H)[:, :, :, DK : DK + 1],
        1.0,
    )
    # per-tile x DMAs: each 256KB completes quickly so LN starts incrementally
    # (one batched DMA would round-robin with the weight transfers and only
    # complete its first bytes' semaphore ~10us later)
    xst = []
    for g in range(4):
        xs = xstage.tile([P, 4, D], F32, tag="xst", name=f"xst_{g}")
        xst.append(xs)
    for t in range(NT):
        nc.sync.dma_start(xst[t // 4][:, t % 4, :], x_d[t * P : (t + 1) * P, :])

    def ln_tile(t, fpsum):
        xt = xst[t // 4][:, t % 4, :]
        st = stats.tile([P, 6], F32, tag="st")
        nc.vector.bn_stats(out=st, in_=xt)
        mv = stats.tile([P, 2], F32, tag="mv")
        nc.vector.bn_aggr(out=mv, in_=st)
        sd = stats.tile([P, 1], F32, tag="sd")
        nc.scalar.activation(
            out=sd, in_=mv[:, 1:2], func=mybir.ActivationFunctionType.Sqrt,
            bias=eps_sb,
        )
        rr = stats.tile([P, 1], F32, tag="rr")
        nc.vector.reciprocal(out=rr, in_=sd)
        xn = xnp.tile([P, D], BF16, tag="xn")
        nc.vector.tensor_scalar(
            out=xn, in0=xt, scalar1=mv[:, 0:1], scalar2=rr,
            op0=mybir.AluOpType.subtract, op1=mybir.AluOpType.mult,
        )
        pt4 = fpsum.tile([P, D], BF16, tag="lnps")
        for c in range(DC):
            nc.tensor.transpose(
                pt4[:, c * P : (c + 1) * P], xn[:, c * P : (c + 1) * P], ident
            )
        # bf16 PSUM reads are 2x on DVE; later tiles aren't on the ACT
        # critical prefix, so they evacuate there
        if t < 8:
            nc.scalar.copy(
                out=xnT[:, :, t * P : (t + 1) * P],
                in_=pt4[:].rearrange("p (c q) -> p c q", c=DC),
            )
        else:
            nc.vector.tensor_copy(
                out=xnT[:, :, t * P : (t + 1) * P],
                in_=pt4[:].rearrange("p (c q) -> p c q", c=DC),
            )

    # ---------------- front: LN, transposes, projections ----------------
    with tc.tile_pool(name="fpsum", bufs=4, space="PSUM") as fpsum:
        for t in range(8):
            ln_tile(t, fpsum)

        # QT projection (queries = tokens 0..NQ-1), evac on ACT into the
        # zero-padded per-head layout (d-chunk dqc = head pair dqc)
        for dqc in range(DC):
            for qg in range(NQ // 512):
                ps = fpsum.tile([P, 512], F32, tag="ppsum")
                for dc in range(DC):
                    nc.tensor.matmul(
                        ps,
                        w_sb["wq"][:, dc, dqc * P : (dqc + 1) * P],
                        xnT[:, dc, qg * 512 : (qg + 1) * 512],
                        start=(dc == 0), stop=(dc == DC - 1),
                    )
                sl = slice(qg * 512, (qg + 1) * 512)
                if use_bias:
                    nc.vector.tensor_scalar_add(
                        out=qTp[0:DK, dqc, 0, sl], in0=ps[0:DK, :],
                        scalar1=bq_sb[0:DK, dqc : dqc + 1],
                    )
                    nc.vector.tensor_scalar_add(
                        out=qTp[DK:P, dqc, 1, sl], in0=ps[DK:P, :],
                        scalar1=bq_sb[DK:P, dqc : dqc + 1],
                    )
                else:
                    nc.scalar.copy(out=qTp[0:DK, dqc, 0, sl], in_=ps[0:DK, :])
                    nc.scalar.copy(out=qTp[DK:P, dqc, 1, sl], in_=ps[DK:P, :])

        # KT projection chunks 0..7, then rest of LN, then chunks 8..15.
        # kg-outer so early key chunks are ready first; evac on DVE.
        def kt_group(kg):
            for dkc in range(DC):
                ps = fpsum.tile([P, 512], F32, tag="ppsum")
                for dc in range(DC):
                    nc.tensor.matmul(
                        ps,
                        w_sb["wk"][:, dc, dkc * P : (dkc + 1) * P],
                        xnT[:, dc, kg * 512 : (kg + 1) * 512],
                        start=(dc == 0), stop=(dc == DC - 1),
                    )
                dst = kT[:, dkc, kg * 512 : (kg + 1) * 512]
                if use_bias:
                    nc.vector.tensor_scalar_add(
                        out=dst, in0=ps, scalar1=bk_sb[:, dkc : dkc + 1]
                    )
                else:
                    nc.vector.tensor_copy(out=dst, in_=ps)

        kt_group(0)
        for t in range(8, 12):
            ln_tile(t, fpsum)
        kt_group(1)
        for t in range(12, 16):
            ln_tile(t, fpsum)
        kt_group(2)
        kt_group(3)

        # preload the exp ACT table while the PE is busy with projections so
        # the first attention exp doesn't eat the ~2.7us table switch.
        dummy = stats.tile([P, 1], F32, tag="dummy")
        nc.scalar.activation(
            out=dummy, in_=eps_sb, func=mybir.ActivationFunctionType.Exp
        )

    # ---------------- attention ----------------
    with (
        tc.tile_pool(name="scp", bufs=2, space="PSUM") as scp,
        tc.tile_pool(name="pvp", bufs=3, space="PSUM") as pvp,
        tc.tile_pool(name="tpp", bufs=1, space="PSUM") as tpp,
    ):
        def evac_pair(p, pvb):
            # evacuate + normalize this pair's slice of attno
            for qt in range(NQT):
                bank = pvb[qt // 3]
                off = (qt % 3) * 130
                rl = rlp.tile([P, 2], F32, tag="rl")
                for hs in range(2):
                    nc.vector.reciprocal(
                        out=rl[:, hs : hs + 1],
                        in_=bank[:, off + hs * 65 + DK : off + hs * 65 + DK + 1],
                    )
                for hs in range(2):
                    nc.vector.tensor_scalar_mul(
                        out=attno[:, qt, (2 * p + hs) * DK : (2 * p + hs + 1) * DK],
                        in0=bank[:, off + hs * 65 : off + hs * 65 + DK],
                        scalar1=rl[:, hs : hs + 1],
                    )

        def transpose_pair(p):
            for qt in range(NQT):
                tt = tpp.tile([P, P], BF16, tag="vps", name=f"tt{p}_{qt}")
                nc.tensor.transpose(
                    tt, attno[:, qt, p * P : (p + 1) * P], ident
                )
                nc.vector.tensor_copy(
                    out=outT[:, p, qt * P : (qt + 1) * P], in_=tt
                )

        def v_group(t):
            # V projection for token tile t, woven into pair 0 so its ACT
            # evacuation fills exp-starvation gaps instead of delaying the
            # first exp; token-major, interleaved [V_h | 1] per head
            ps = tpp.tile([P, 512], F32, tag="vps", name=f"vps{t}")
            for dc in range(DC):
                nc.tensor.matmul(
                    ps,
                    xnT[:, dc, t * P : (t + 1) * P],
                    w_sb["wv"][:, dc, :],
                    start=(dc == 0), stop=(dc == DC - 1),
                )
            vslot = vaug[:, t, :].rearrange("p (h c) -> p h c", h=H)
            nc.scalar.copy(
                out=vslot[:, :, 0:DK],
                in_=ps[:].rearrange("p (h c) -> p h c", h=H),
            )

        prev = None  # (p, pvb) awaiting finalization
        for p in range(PAIRS):
            pvb = [
                pvp.tile([P, 512], F32, tag="pvb", name=f"pvb{p}_{j}")
                for j in range(3)
            ]
            pts = []
            for c in range(NT):
                scs = [scp.tile([P, NQ], F32, tag="sc", name=f"sc{p}_{c}_{h}")
                       for h in range(2)]
                for hs in range(2):
                    for qg in range(NQ // 512):
                        nc.tensor.matmul(
                            scs[hs][:, qg * 512 : (qg + 1) * 512],
                            kT[:, p, c * P : (c + 1) * P],
                            qTp[:, p, hs, qg * 512 : (qg + 1) * 512],
                            start=True, stop=True,
                        )
                pt = ptp.tile([P, 2 * NQ], BF16, tag="pt")
                pts.append(pt)
                for hs in range(2):
                    nc.scalar.activation(
                        out=pt[:, hs * NQ : (hs + 1) * NQ], in_=scs[hs],
                        func=mybir.ActivationFunctionType.Exp,
                        bias=mb_sb[:, c : c + 1], scale=1.0 / math.sqrt(DK),
                    )
                # pair 0: weave the V projection, two token tiles per chunk
                if p == 0 and c < 8:
                    v_group(2 * c)
                    v_group(2 * c + 1)
                # finalize the previous pair after this pair's pipeline is
                # primed, so its DVE/PE work hides under our exps
                if c == 1 and prev is not None:
                    evac_pair(*prev)
                if c == 2 and prev is not None:
                    transpose_pair(prev[0])
                    prev = None
                # PV matmuls for the previous chunk (keeps PE busy while ACT
                # works on this chunk's exp)
                if c > 0:
                    _pv_chunk(nc, pts[c - 1], vaug, pvb, p, c - 1)
            _pv_chunk(nc, pts[NT - 1], vaug, pvb, p, NT - 1)
            prev = (p, pvb)
        evac_pair(*prev)
        transpose_pair(prev[0])

    # ---------------- output projection ----------------
    with tc.tile_pool(name="opsum", bufs=3, space="PSUM") as opsum:
        for qt in range(NQT):
            po = opsum.tile([P, D], F32, tag="epsum")
            for dc in range(DC):
                nc.tensor.matmul(
                    po,
                    outT[:, dc, qt * P : (qt + 1) * P],
                    w_sb["wo"][:, dc, :],
                    start=(dc == 0), stop=(dc == DC - 1),
                )
            yt = yout.tile([P, D], F32, tag="yt")
            if use_bias:
                nc.vector.tensor_tensor(
                    out=yt, in0=po, in1=bo_sb, op=mybir.AluOpType.add
                )
            elif qt % 2 == 0:
                nc.vector.tensor_copy(out=yt, in_=po)
            else:
                nc.scalar.copy(out=yt, in_=po)
            nc.sync.dma_start(y_d[qt * P : (qt + 1) * P, :], yt)


def _pv_chunk(nc, pt, vaug, pvb, p, c):
    """P@[V|1] matmuls for chunk c of head-pair p: 8 query tiles x 2 heads,
    accumulated over chunks into the packed PSUM banks."""
    for qt in range(NQT):
        bank = pvb[qt // 3]
        off = (qt % 3) * 130
        for hs in range(2):
            h = 2 * p + hs
            # start=True clears has_written for the WHOLE bank, so only the
            # first packed region per bank may use it; the others rely on
            # overwrite-when-bit-clear for their first chunk.
            nc.tensor.matmul(
                bank[:, off + hs * 65 : off + (hs + 1) * 65],
                pt[:, hs * NQ + qt * P : hs * NQ + (qt + 1) * P],
                vaug[:, c, h * 65 : (h + 1) * 65],
                start=(c == 0 and qt % 3 == 0 and hs == 0),
                stop=(c == NT - 1),
                skip_group_check=True,
            )


_NC = {}


def _get_nc(use_bias: bool):
    if use_bias not in _NC:
        from contextlib import ExitStack

        nc = bacc.Bacc(None, target_bir_lowering=False)
        with tile.TileContext(nc) as tc, ExitStack() as ctx:
            _emit(tc, ctx, use_bias)
        nc.compile()
        _NC[use_bias] = nc
    return _NC[use_bias]


def kernel(
    inputs, input_lengths, pos_embed, ln_gamma, ln_beta,
    Wq, bq, Wk, bk, Wv, bv, Wo, bo,
):
    x = np.ascontiguousarray(np.asarray(inputs, np.float32))
    lengths = np.asarray(input_lengths, np.int32)
    g = np.asarray(ln_gamma, np.float32)
    be = np.asarray(ln_beta, np.float32)
    Wq = np.asarray(Wq, np.float32); bq = np.asarray(bq, np.float32)
    Wk = np.asarray(Wk, np.float32); bk = np.asarray(bk, np.float32)
    Wv = np.asarray(Wv, np.float32); bv = np.asarray(bv, np.float32)
    Wo = np.asarray(Wo, np.float32); bo = np.asarray(bo, np.float32)

    import ml_dtypes

    bf16 = ml_dtypes.bfloat16
    # Fold LayerNorm affine into the projections (exact: LN(x) = xh*g + be
    # with xh = (x-mu)*rstd, so LN(x)@W.T + b = xh@(g[:,None]*W.T) + (be@W.T + b)).
    # Weights ship as bf16 (the kernel computes in bf16 anyway) so the device
    # skips the fp32 staging + cast entirely.
    wq_h = np.ascontiguousarray((g[:, None] * Wq.T).astype(bf16))
    wk_h = np.ascontiguousarray((g[:, None] * Wk.T).astype(bf16))
    wv_h = np.ascontiguousarray((g[:, None] * Wv.T).astype(bf16))
    wo_h = np.ascontiguousarray(Wo.T.astype(bf16))
    bq_f = be @ Wq.T + bq
    bk_f = be @ Wk.T + bk
    # V bias (incl. beta term) passes through softmax (rows sum to 1) and is
    # folded into the output-projection bias.
    bv_h = be @ Wv.T + bv
    bo_h = np.ascontiguousarray(bo + bv_h @ Wo.T)

    use_bias = bool(
        np.any(bq_f) or np.any(bk_f) or np.any(bo_h)
    )

    maskb = np.where(np.arange(S)[None, :] < lengths[:, None], ESHIFT, NEG).astype(
        np.float32
    )

    nc = _get_nc(use_bias)
    in_maps = []
    core_assign = []
    for b in range(B):
        for gq in range(2):
            order = np.r_[gq * NQ : (gq + 1) * NQ, (1 - gq) * NQ : (2 - gq) * NQ]
            im = {
                "x": np.ascontiguousarray(x[b][order]),
                "wq": wq_h, "wk": wk_h, "wv": wv_h, "wo": wo_h,
                "maskb": np.ascontiguousarray(maskb[b][order].reshape(NT, P).T),
            }
            if use_bias:
                im["bq"] = np.ascontiguousarray(bq_f.reshape(DC, P).T)
                im["bk"] = np.ascontiguousarray(bk_f.reshape(DC, P).T)
                im["bo"] = bo_h
            in_maps.append(im)
            core_assign.append((b, gq))

    global _LAST_IN_MAPS, _LAST_NC
    _LAST_IN_MAPS = in_maps
    _LAST_NC = nc
    res = run_bass_kernel_spmd(nc, in_maps, core_ids=list(range(8)))

    y = np.empty((B, S, D), np.float32)
    for i, (b, gq) in enumerate(core_assign):
        y[b, gq * NQ : (gq + 1) * NQ] = res.results[i]["y"]
    return y
